# revision 1
# baseline (speedup 1.0000x reference)
"""EvolveGCNO RecurrentGCN forward on 8 trn2 NeuronCores.

Strategy (dst-sharded gather, v2):
  - Nodes sharded by destination across 8 cores (6250 each, padded to 6272).
    Edges live on the core owning their dst; self-loops are ordinary edges.
  - Phase A (device): per-core degree via padded-CSR row sums, dinv =
    1/sqrt(deg); scale own x rows by dinv -> xs; AllGather xs so every core
    holds the full scaled feature table (this removes any per-edge dinv[src]
    work: gathered rows already carry dinv[src]).
  - GRU weight evolution on device (replicated on every core).
  - Phase B (device): per block of 512 dsts, dma_gather xs[src] rows (512 B
    descriptors - the memory-roofline term) from the two int16-addressable
    halves of the table; build one-hot scatter matrices
    S[e, j] = (iota == off) * ew with one chained tensor_scalar per 128-edge
    chunk; aggregate with PE matmuls into PSUM [128f, BLK d]; scale by
    dinv[dst] (partition-broadcast DMA), apply evolved W, ReLU, lin_w ->
    per-core output slice.

Host work is limited to graph partitioning / index manipulation / layout
(sorting, bincount, padding, parameter transposes); all floating point math
on tensor values happens on device.
"""

import math
import sys

import numpy as np

sys.path.insert(0, "/opt/trn_rl_repo")

N_NODES, N_EDGES, C = 50000, 600000, 128
NCORES = 8
NPC = N_NODES // NCORES            # 6250 nodes per core
NTILE = (NPC + 127) // 128         # 49 sbuf tiles of 128 nodes
NPAD = NTILE * 128                 # 6272 padded nodes per core
HALFL = NPAD // 2                  # 3136: per-core split row for the tables
HALF = NCORES * HALFL              # 25088 rows per gather table (int16-safe)
WDST = 128                         # dsts per psum column window
NWINDOW = NPAD // WDST             # 49 windows per core
BLK = 512                          # dsts per psum block
WPB = BLK // WDST                  # full-block windows (4)
NBLK = (NPAD + BLK - 1) // BLK     # 13 blocks (last block: 1 window)


# ---------------------------------------------------------------------------
# Host-side preprocessing: graph partitioning + layout (index work only)
# ---------------------------------------------------------------------------

def preprocess(edge_index: np.ndarray, edge_weight: np.ndarray):
    src = np.asarray(edge_index[0], dtype=np.int64)
    dst = np.asarray(edge_index[1], dtype=np.int64)
    ew = np.asarray(edge_weight, dtype=np.float32)

    loop = np.arange(N_NODES, dtype=np.int64)
    src_a = np.concatenate([src, loop])
    dst_a = np.concatenate([dst, loop])
    ew_a = np.concatenate([ew, np.ones(N_NODES, np.float32)])

    core_of = dst_a // NPC
    percore = []
    kl = kh = 1
    kdeg = 1
    for c in range(NCORES):
        m = core_of == c
        s = src_a[m]
        l = (dst_a[m] - c * NPC).astype(np.int64)
        w = ew_a[m]
        lsrc = s % NPC
        half = (lsrc >= HALFL).astype(np.int64)
        idx16 = ((s // NPC) * HALFL + (lsrc - half * HALFL)).astype(np.int64)
        # group edges by (window, half), dst-sorted within each group
        key = (l // WDST) * 2 + half
        order = np.argsort(key * (NPC + 1) + l, kind="stable")
        l, w, half, idx16 = l[order], w[order], half[order], idx16[order]
        win = l // WDST
        percore.append((l, w, half, idx16, win))
        for h in (0, 1):
            cnt = np.bincount(win[half == h], minlength=NWINDOW)
            k = int(math.ceil(cnt.max() / 128))
            if h == 0:
                kl = max(kl, k)
            else:
                kh = max(kh, k)
        kdeg = max(kdeg, int(np.bincount(l, minlength=NPAD).max()))

    KL, KH, KDEG = kl, kh, kdeg
    nw_b = [min(WPB, NWINDOW - WPB * b) for b in range(NBLK)]
    blk_start = np.cumsum([0] + [nw * (KL + KH) for nw in nw_b])
    TOT = int(blk_start[-1])                       # total meta columns

    metas = []
    for c in range(NCORES):
        l, w, half, idx16, win = percore[c]
        wb = win % WPB                              # window within block
        b = win // WPB
        nw = np.array(nw_b)[b]
        # position within the (window, half) group
        grp = win * 2 + half
        gcnt = np.bincount(grp, minlength=2 * NWINDOW)
        gstart = np.cumsum(gcnt) - gcnt
        p_in = np.arange(len(l)) - gstart[grp]
        j = p_in // 128
        row = p_in % 128
        col = np.where(
            half == 0,
            blk_start[b] + wb * KL + j,
            blk_start[b] + nw * KL + wb * KH + j,
        )

        offv = np.zeros((128, TOT), np.float32)
        ewv = np.zeros((128, TOT), np.float32)
        offv[row, col] = (l % WDST).astype(np.float32)
        ewv[row, col] = w

        # gather index lists, one per (block, half), packed along columns.
        # list position i = (col_rel * 128 + row); idx 0 pads (killed by ew=0)
        idxlo = np.zeros((128, TOT * 8), np.int16)   # upper bound on columns
        idxhi = np.zeros((128, TOT * 8), np.int16)
        lo_starts, hi_starts = [], []
        lo_c = hi_c = 0
        for bb in range(NBLK):
            nwb = nw_b[bb]
            lo_starts.append(lo_c)
            hi_starts.append(hi_c)
            lo_c += nwb * KL * 8
            hi_c += nwb * KH * 8
        CL, CH = lo_c, hi_c
        idxlo = np.zeros((16, CL), np.int16)
        idxhi = np.zeros((16, CH), np.int16)
        for h, (arr, starts, KX, nlo) in enumerate(
                [(idxlo, lo_starts, KL, True), (idxhi, hi_starts, KH, False)]):
            mh = half == h
            bb = b[mh]
            col_rel = wb[mh] * KX + j[mh]           # position within half
            i_list = col_rel * 128 + row[mh]        # position in block's list
            ci = np.array(starts)[bb] * 16 + i_list  # global flat position
            arr[ci % 16, ci // 16] = idx16[mh]
        metas.append(dict(
            offv=offv, ewv=ewv,
            idxlo=np.ascontiguousarray(np.tile(idxlo, (8, 1))),
            idxhi=np.ascontiguousarray(np.tile(idxhi, (8, 1))),
        ))

    # padded CSR of edge weights for the degree computation
    for c in range(NCORES):
        l, w, half, idx16, win = percore[c]
        counts = np.bincount(l, minlength=NPAD)
        starts = np.cumsum(counts) - counts
        # l is sorted by (win, half, l); slot within dst needs a stable
        # position per dst: order by l
        o2 = np.argsort(l, kind="stable")
        ls, ws = l[o2], w[o2]
        slot = np.arange(len(ls)) - starts[ls]
        csr = np.zeros((NPAD, KDEG), np.float32)
        csr[ls, slot] = ws
        csr[NPC:, 0] = 1.0
        metas[c]["csr"] = csr

    pre = dict(KL=KL, KH=KH, KDEG=KDEG, TOT=TOT, nw_b=nw_b,
               blk_start=[int(v) for v in blk_start],
               lo_starts=lo_starts, hi_starts=hi_starts, CL=CL, CH=CH)
    return pre, metas


def make_in_maps(inp: dict, pre, metas):
    iota = np.tile(np.arange(WDST, dtype=np.float32), (128, 1))
    W0 = np.asarray(inp["W0"], np.float32)
    x = np.ascontiguousarray(np.asarray(inp["x"], np.float32))
    shared = dict(
        iota=np.ascontiguousarray(iota),
        w0=W0,
        w0t=np.ascontiguousarray(W0.T),
        wiht=np.ascontiguousarray(np.asarray(inp["gru_w_ih"], np.float32).T),
        whht=np.ascontiguousarray(np.asarray(inp["gru_w_hh"], np.float32).T),
        bih=np.asarray(inp["gru_b_ih"], np.float32),
        bhh=np.asarray(inp["gru_b_hh"], np.float32),
        linw=np.ascontiguousarray(np.asarray(inp["lin_w"], np.float32).T),
        linb=np.asarray(inp["lin_b"], np.float32).reshape(1, 1),
    )
    maps = []
    for c in range(NCORES):
        xo = np.zeros((NPAD, C), np.float32)
        xo[:NPC] = x[c * NPC:(c + 1) * NPC]
        maps.append(dict(shared, x_own=xo, **metas[c]))
    return maps


# ---------------------------------------------------------------------------
# Device program
# ---------------------------------------------------------------------------

def build_program(pre, debug_taps: bool = False, skip_gather: bool = False,
                  skip_collective: bool = False, nblk: int = NBLK,
                  rep: int = 1, skip_compute: bool = False,
                  gather_elem: int = C, nqueues: int = 4, fp16: bool = False,
                  dma_scratch: int = 16384):
    import concourse.bacc as bacc
    import concourse.bass as bass
    import concourse.tile as tile
    from concourse import mybir

    f32 = mybir.dt.float32
    f16 = mybir.dt.float16 if fp16 else mybir.dt.float32
    i16 = mybir.dt.int16
    AF = mybir.ActivationFunctionType
    OP = mybir.AluOpType
    KL, KH, KDEG, TOT = pre["KL"], pre["KH"], pre["KDEG"], pre["TOT"]
    nw_b = pre["nw_b"]
    blk_start = pre["blk_start"]
    lo_starts, hi_starts = pre["lo_starts"], pre["hi_starts"]
    CL, CH = pre["CL"], pre["CH"]
    MAXCOL = WPB * (KL + KH)          # widest block in meta columns

    nc = bacc.Bacc("TRN2", target_bir_lowering=False, debug=False,
                   num_devices=NCORES, num_swdge_queues=nqueues,
                   dynamic_dma_scratch_size=dma_scratch)

    x_own_t = nc.declare_dram_parameter("x_own", [NPAD, C], f32, isOutput=False)
    idxlo_t = nc.declare_dram_parameter("idxlo", [128, CL], i16, isOutput=False)
    idxhi_t = nc.declare_dram_parameter("idxhi", [128, CH], i16, isOutput=False)
    offv_t = nc.declare_dram_parameter("offv", [128, TOT], f32, isOutput=False)
    ewv_t = nc.declare_dram_parameter("ewv", [128, TOT], f32, isOutput=False)
    csr_t = nc.declare_dram_parameter("csr", [NPAD, KDEG], f32, isOutput=False)
    iota_t = nc.declare_dram_parameter("iota", [128, WDST], f32, isOutput=False)
    w0_t = nc.declare_dram_parameter("w0", [C, C], f32, isOutput=False)
    w0t_t = nc.declare_dram_parameter("w0t", [C, C], f32, isOutput=False)
    wiht_t = nc.declare_dram_parameter("wiht", [C, 3 * C], f32, isOutput=False)
    whht_t = nc.declare_dram_parameter("whht", [C, 3 * C], f32, isOutput=False)
    bih_t = nc.declare_dram_parameter("bih", [3 * C], f32, isOutput=False)
    bhh_t = nc.declare_dram_parameter("bhh", [3 * C], f32, isOutput=False)
    linw_t = nc.declare_dram_parameter("linw", [C, 1], f32, isOutput=False)
    linb_t = nc.declare_dram_parameter("linb", [1, 1], f32, isOutput=False)
    out_t = nc.declare_dram_parameter("out", [NPAD], f32, isOutput=True)

    dinv_own_hbm = nc.dram_tensor("dinv_own_hbm", [NPAD], f32)
    xs_own_hbm = nc.dram_tensor("xs_own_hbm", [NPAD, C], f16)
    xs_allA = nc.dram_tensor("xs_allA", [HALF, C], f16, addr_space="Shared")
    xs_allB = nc.dram_tensor("xs_allB", [HALF, C], f16, addr_space="Shared")
    dbg = {}
    if debug_taps:
        dbg["g0"] = nc.declare_dram_parameter(
            "dbg_g0", [128, MAXCOL * C], f32, isOutput=True)
        dbg["s0"] = nc.declare_dram_parameter("dbg_s0", [128, WDST], f32,
                                              isOutput=True)
        dbg["u0"] = nc.declare_dram_parameter("dbg_u0", [128, BLK], f32,
                                              isOutput=True)
        dbg["xsall"] = nc.declare_dram_parameter(
            "dbg_xsall", [128, C], f32, isOutput=True)

    def bcast_partitions(ap, parts=128):
        return bass.AP(tensor=ap.tensor, offset=ap.offset,
                       ap=[[0, parts]] + list(ap.ap))

    with tile.TileContext(nc) as tc:
        with (
            tc.tile_pool(name="singles", bufs=1) as singles,
            tc.tile_pool(name="gru", bufs=1) as gru,
            tc.tile_pool(name="gpool", bufs=2) as gpool,
            tc.tile_pool(name="spool", bufs=16) as spool,
            tc.tile_pool(name="mpool", bufs=3) as mpool,
            tc.tile_pool(name="upool", bufs=3) as upool,
            tc.tile_pool(name="opool", bufs=3) as opool,
            tc.tile_pool(name="pagg", bufs=2, space="PSUM") as pagg,
            tc.tile_pool(name="ph", bufs=2, space="PSUM") as ph,
            tc.tile_pool(name="po", bufs=2, space="PSUM") as po,
            tc.tile_pool(name="pjunk", bufs=1, space="PSUM") as pjunk,
        ):
            junk_ps = pjunk.tile([1, 1], f32, tag="junk")

            def pe_absorb(ap):
                nc.tensor.matmul(junk_ps[:1, :1], lhsT=ap, rhs=ap,
                                 start=True, stop=True)

            # ---------------- constants / metadata loads ----------------
            iota_sb = singles.tile([128, WDST], f32)
            nc.sync.dma_start(iota_sb[:], iota_t[:])
            idxlo_sb = singles.tile([128, CL], i16)
            nc.sync.dma_start(idxlo_sb[:], idxlo_t[:])
            idxhi_sb = singles.tile([128, CH], i16)
            nc.sync.dma_start(idxhi_sb[:], idxhi_t[:])
            offv_sb = singles.tile([128, TOT], f32)
            nc.sync.dma_start(offv_sb[:], offv_t[:])
            ewv_sb = singles.tile([128, TOT], f32)
            nc.sync.dma_start(ewv_sb[:], ewv_t[:])
            linw_sb = singles.tile([C, 1], f32)
            nc.sync.dma_start(linw_sb[:], linw_t[:])
            pe_absorb(linw_sb[:1, :1])
            linb_sb = singles.tile([1, 1], f32)
            nc.sync.dma_start(linb_sb[:], linb_t[:])

            # ---------------- phase A: deg -> dinv -> xs -> allgather ----
            csr_sb = singles.tile([128, NTILE, KDEG], f32)
            nc.sync.dma_start(csr_sb[:],
                              csr_t[:].rearrange("(t r) k -> r t k", r=128))
            deg_sb = singles.tile([128, NTILE], f32)
            for t in range(NTILE):
                nc.vector.reduce_sum(deg_sb[:, t:t + 1], csr_sb[:, t, :],
                                     axis=mybir.AxisListType.X)
            sqrt_sb = singles.tile([128, NTILE], f32)
            nc.scalar.activation(sqrt_sb[:], deg_sb[:], AF.Sqrt)
            dinv_sb = singles.tile([128, NTILE], f32)
            nc.vector.reciprocal(dinv_sb[:], sqrt_sb[:])
            nc.sync.dma_start(
                dinv_own_hbm[:].rearrange("(t r) -> r t", r=128), dinv_sb[:])

            xown_sb = singles.tile([128, NTILE, C], f32)
            nc.sync.dma_start(
                xown_sb[:], x_own_t[:].rearrange("(t r) f -> r t f", r=128))
            xs_sb = singles.tile([128, NTILE, C], f16)
            for t in range(NTILE):
                nc.vector.tensor_scalar(
                    out=xs_sb[:, t, :], in0=xown_sb[:, t, :],
                    scalar1=dinv_sb[:, t:t + 1], scalar2=None, op0=OP.mult)
            nc.sync.dma_start(
                xs_own_hbm[:].rearrange("(t r) f -> r t f", r=128), xs_sb[:])
            if skip_collective:
                nc.sync.dma_start(xs_allA[:HALFL, :], xs_own_hbm[:HALFL, :])
                nc.sync.dma_start(xs_allB[:HALFL, :], xs_own_hbm[HALFL:, :])
            else:
                nc.gpsimd.collective_compute(
                    "AllGather",
                    OP.bypass,
                    replica_groups=[list(range(NCORES))],
                    ins=[xs_own_hbm[:HALFL, :].opt()],
                    outs=[xs_allA[:].opt()],
                )
                nc.gpsimd.collective_compute(
                    "AllGather",
                    OP.bypass,
                    replica_groups=[list(range(NCORES))],
                    ins=[xs_own_hbm[HALFL:, :].opt()],
                    outs=[xs_allB[:].opt()],
                )


            # ---------------- GRU weight evolution ----------------------
            w0_sb = gru.tile([C, C], f32)
            nc.sync.dma_start(w0_sb[:], w0_t[:])
            w0t_sb = gru.tile([C, C], f32)
            nc.sync.dma_start(w0t_sb[:], w0t_t[:])
            pe_absorb(w0t_sb[:1, :1])
            wiht_sb = gru.tile([C, 3 * C], f32)
            nc.sync.dma_start(wiht_sb[:], wiht_t[:])
            whht_sb = gru.tile([C, 3 * C], f32)
            nc.sync.dma_start(whht_sb[:], whht_t[:])
            bihb_sb = gru.tile([128, 3 * C], f32)
            nc.gpsimd.dma_start(out=bihb_sb[:], in_=bcast_partitions(bih_t[:]))
            bhhb_sb = gru.tile([128, 3 * C], f32)
            nc.gpsimd.dma_start(out=bhhb_sb[:], in_=bcast_partitions(bhh_t[:]))

            gx_ps = pagg.tile([128, 3 * C], f32, tag="agg_ps")
            nc.tensor.matmul(gx_ps[:], lhsT=w0t_sb[:], rhs=wiht_sb[:],
                             start=True, stop=True)
            gxb = gru.tile([128, 3 * C], f32)
            nc.vector.tensor_tensor(out=gxb[:], in0=gx_ps[:], in1=bihb_sb[:],
                                    op=OP.add)
            gh_ps = pagg.tile([128, 3 * C], f32, tag="agg_ps")
            nc.tensor.matmul(gh_ps[:], lhsT=w0t_sb[:], rhs=whht_sb[:],
                             start=True, stop=True)
            ghb = gru.tile([128, 3 * C], f32)
            nc.vector.tensor_tensor(out=ghb[:], in0=gh_ps[:], in1=bhhb_sb[:],
                                    op=OP.add)
            rz_in = gru.tile([128, 2 * C], f32)
            nc.vector.tensor_tensor(out=rz_in[:], in0=gxb[:, :2 * C],
                                    in1=ghb[:, :2 * C], op=OP.add)
            rz = gru.tile([128, 2 * C], f32)
            nc.scalar.activation(rz[:], rz_in[:], AF.Sigmoid)
            t1 = gru.tile([128, C], f32)
            nc.vector.tensor_tensor(out=t1[:], in0=rz[:, :C],
                                    in1=ghb[:, 2 * C:], op=OP.mult)
            t2 = gru.tile([128, C], f32)
            nc.vector.tensor_tensor(out=t2[:], in0=gxb[:, 2 * C:], in1=t1[:],
                                    op=OP.add)
            n_sb = gru.tile([128, C], f32)
            nc.scalar.activation(n_sb[:], t2[:], AF.Tanh)
            d_sb = gru.tile([128, C], f32)
            nc.vector.tensor_tensor(out=d_sb[:], in0=w0_sb[:], in1=n_sb[:],
                                    op=OP.subtract)
            e_sb = gru.tile([128, C], f32)
            nc.vector.tensor_tensor(out=e_sb[:], in0=rz[:, C:], in1=d_sb[:],
                                    op=OP.mult)
            w_sb = gru.tile([C, C], f32)
            nc.vector.tensor_tensor(out=w_sb[:], in0=n_sb[:], in1=e_sb[:],
                                    op=OP.add)

            # ---------------- phase B: gather + aggregate ----------------
            blk_list = [bb for _ in range(rep) for bb in range(nblk)]
            gtiles = {}

            def issue_block_loads(bi):
                b = blk_list[bi]
                nw = nw_b[b]
                ncol = nw * (KL + KH)
                nd = nw * WDST
                g_sb = gpool.tile([128, MAXCOL, gather_elem], f16, tag="g")
                if not skip_gather:
                    nc.gpsimd.dma_gather(
                        g_sb[:, :nw * KL, :],
                        xs_allA[:, :gather_elem],
                        idxlo_sb[:, lo_starts[b]:lo_starts[b] + nw * KL * 8],
                        nw * KL * 128,
                        nw * KL * 128,
                        gather_elem,
                        elem_step=C,
                        single_packet=False,
                        queue_num=(2 * b) % nqueues,
                    )
                    nc.gpsimd.dma_gather(
                        g_sb[:, nw * KL:ncol, :],
                        xs_allB[:, :gather_elem],
                        idxhi_sb[:, hi_starts[b]:hi_starts[b] + nw * KH * 8],
                        nw * KH * 128,
                        nw * KH * 128,
                        gather_elem,
                        elem_step=C,
                        single_packet=False,
                        queue_num=(2 * b + 1) % nqueues,
                    )
                else:
                    nc.gpsimd.memset(g_sb[:, :1, :], 0.0)
                dinvb_sb = mpool.tile([128, BLK], f32, tag="dinvb")
                nc.gpsimd.dma_start(
                    out=dinvb_sb[:, :nd],
                    in_=bcast_partitions(dinv_own_hbm[BLK * b:BLK * b + nd]))
                gtiles[bi] = (g_sb, dinvb_sb)

            issue_block_loads(0)
            if len(blk_list) > 1:
                issue_block_loads(1)
            for bi, b in enumerate(blk_list):
                nw = nw_b[b]
                ncol = nw * (KL + KH)
                c0 = blk_start[b]
                nd = nw * WDST
                g_sb, dinvb_sb = gtiles.pop(bi)
                pe_absorb(g_sb[:1, :1, :1])
                if bi + 2 < len(blk_list):
                    issue_block_loads(bi + 2)

                agg_ps = pagg.tile([128, BLK], f32, tag="agg_ps")
                if skip_compute:
                    nc.vector.memset(agg_ps[:, :nd], 0.0)
                for w in range(nw):
                    if skip_compute:
                        break
                    for h, KX in ((0, KL), (1, KH)):
                        for j in range(KX):
                            col = (w * KL + j) if h == 0 else (
                                nw * KL + w * KH + j)
                            s_sb = spool.tile([128, WDST], f16, tag="s")
                            nc.vector.tensor_scalar(
                                out=s_sb[:],
                                in0=iota_sb[:],
                                scalar1=offv_sb[:, c0 + col:c0 + col + 1],
                                scalar2=ewv_sb[:, c0 + col:c0 + col + 1],
                                op0=OP.is_equal,
                                op1=OP.mult,
                            )
                            if debug_taps and b == 0 and w == 0 and h == 0 \
                                    and j == 0:
                                nc.sync.dma_start(dbg["s0"][:], s_sb[:])
                            nc.tensor.matmul(
                                agg_ps[:, w * WDST:(w + 1) * WDST],
                                lhsT=g_sb[:, col, :],
                                rhs=s_sb[:],
                                start=(h == 0 and j == 0),
                                stop=(h == 1 and j == KH - 1),
                            )

                u_sb = upool.tile([128, BLK], f32, tag="u")
                nc.vector.tensor_tensor(out=u_sb[:, :nd], in0=agg_ps[:, :nd],
                                        in1=dinvb_sb[:, :nd], op=OP.mult)
                if debug_taps and b == 0:
                    nc.sync.dma_start(dbg["u0"][:], u_sb[:])
                h_ps = ph.tile([128, BLK], f32, tag="h")
                nc.tensor.matmul(h_ps[:, :nd], lhsT=w_sb[:], rhs=u_sb[:, :nd],
                                 start=True, stop=True)
                r_sb = upool.tile([128, BLK], f32, tag="r")
                nc.vector.tensor_scalar(
                    out=r_sb[:, :nd], in0=h_ps[:, :nd], scalar1=0.0,
                    scalar2=None, op0=OP.max)
                o_ps = po.tile([1, BLK], f32)
                nc.tensor.matmul(o_ps[:1, :nd], lhsT=linw_sb[:],
                                 rhs=r_sb[:, :nd], start=True, stop=True)
                orow_sb = opool.tile([1, BLK], f32, tag="orow")
                nc.vector.tensor_scalar(
                    out=orow_sb[:1, :nd], in0=o_ps[:1, :nd],
                    scalar1=linb_sb[:1, :1], scalar2=None, op0=OP.add)
                nc.sync.dma_start(out_t[BLK * b:BLK * b + nd],
                                  orow_sb[:1, :nd])
    nc.compile()
    return nc


# ---------------------------------------------------------------------------
# Entry point
# ---------------------------------------------------------------------------

_PROG_CACHE = {}


def kernel(x, edge_index, edge_weight, W0, gru_w_ih, gru_w_hh,
           gru_b_ih, gru_b_hh, lin_w, lin_b):
    from concourse.bass_utils import run_bass_kernel_spmd

    pre, metas = preprocess(np.asarray(edge_index), np.asarray(edge_weight))
    key = (pre["KL"], pre["KH"], pre["KDEG"], pre["TOT"])
    if key not in _PROG_CACHE:
        _PROG_CACHE[key] = build_program(pre)
    nc = _PROG_CACHE[key]
    inp = dict(x=x, W0=W0, gru_w_ih=gru_w_ih, gru_w_hh=gru_w_hh,
               gru_b_ih=gru_b_ih, gru_b_hh=gru_b_hh, lin_w=lin_w, lin_b=lin_b)
    in_maps = make_in_maps(inp, pre, metas)
    res = run_bass_kernel_spmd(nc, in_maps, list(range(NCORES)))
    out = np.concatenate([np.asarray(res.results[c]["out"])[:NPC]
                          for c in range(NCORES)])
    return out.reshape(N_NODES, 1).astype(np.float32)



# revision 3
# speedup vs baseline: 1.4834x; 1.4834x over previous
"""EvolveGCNO RecurrentGCN forward on 8 trn2 NeuronCores.

Strategy (dst-sharded gather, v3 - bf16):
  - Nodes sharded by destination across 8 cores (6250 each, padded to 6400).
    Edges live on the core owning their dst; self-loops are NOT materialized
    as edges - they are applied on-device as identity matmuls over the local
    scaled-feature tiles.
  - Phase A (device): per-core degree via padded-CSR row sums, dinv =
    1/sqrt(deg + 1); scale own x rows by dinv -> xs (bf16); ONE AllGather of
    xs laid out as [3200, 2C] row-pairs (pair = local rows l and l+3200), so
    the full 25600-row table stays int16-indexable and gathers select the
    pair half via a column-offset access pattern.
  - GRU weight evolution on device (replicated on every core).
  - Phase B (device): per block of 512 dsts, dma_gather bf16 xs[src] rows
    (256B descriptors - the memory-roofline term); build one-hot scatter
    matrices S[e, j] = (iota == off) * ew in bf16 (DVE 4x mode) per 128-edge
    chunk; aggregate with bf16 PE matmuls into PSUM [128f, 512d], self-loop
    term added via identity matmuls on xs tiles; then W matmul, ReLU, lin
    row; dinv[dst] is factored past ReLU/lin and applied per-window as a
    [128, 1] per-partition column op.

Host work is limited to graph partitioning / index manipulation / layout
(sorting, bincount, padding, parameter transposes); all floating point math
on tensor values happens on device.
"""

import math
import sys

import numpy as np

sys.path.insert(0, "/opt/trn_rl_repo")

N_NODES, N_EDGES, C = 50000, 600000, 128
NCORES = 8
NPC = N_NODES // NCORES            # 6250 nodes per core
NTILE = 50                         # sbuf tiles of 128 nodes
NPAD = NTILE * 128                 # 6400 padded nodes per core
HALFL = NPAD // 2                  # 3200: per-core pair split
TROWS = NCORES * HALFL             # 25600 rows in the paired gather table
WDST = 128                         # dsts per psum column window
NWINDOW = NPAD // WDST             # 50 windows per core
BLK = 512                          # dsts per psum block
WPB = BLK // WDST                  # full-block windows (4)
NBLK = (NPAD + BLK - 1) // BLK     # 13 blocks (last block: 2 windows)


# ---------------------------------------------------------------------------
# Host-side preprocessing: graph partitioning + layout (index work only)
# ---------------------------------------------------------------------------

def preprocess(edge_index: np.ndarray, edge_weight: np.ndarray):
    src = np.asarray(edge_index[0], dtype=np.int64)
    dst = np.asarray(edge_index[1], dtype=np.int64)
    ew = np.asarray(edge_weight, dtype=np.float32)

    core_of = dst // NPC
    nw_b = [min(WPB, NWINDOW - WPB * b) for b in range(NBLK)]
    percore = []
    kl = kh = 1
    kdeg = 1
    for c in range(NCORES):
        m = core_of == c
        s = src[m]
        l = (dst[m] - c * NPC).astype(np.int64)
        w = ew[m]
        sc = s // NPC
        sl = s - sc * NPC
        half = (sl >= HALFL).astype(np.int64)
        idx16 = (sc * HALFL + (sl - half * HALFL)).astype(np.int64)
        # group edges by (window, half), dst-sorted within each group
        key = (l // WDST) * 2 + half
        order = np.argsort(key * (NPAD + 1) + l, kind="stable")
        l, w, half, idx16 = l[order], w[order], half[order], idx16[order]
        win = l // WDST
        percore.append((l, w, half, idx16, win))
        for h in (0, 1):
            cnt = np.bincount(win[half == h], minlength=NWINDOW)
            k = int(math.ceil(max(int(cnt.max()), 1) / 128))
            if h == 0:
                kl = max(kl, k)
            else:
                kh = max(kh, k)
        kdeg = max(kdeg, int(np.bincount(l, minlength=NPAD).max()))

    KL, KH, KDEG = kl, kh, kdeg
    blk_start = np.cumsum([0] + [nw * (KL + KH) for nw in nw_b])
    TOT = int(blk_start[-1])                       # total meta columns

    lo_starts, hi_starts = [], []
    lo_c = hi_c = 0
    for bb in range(NBLK):
        nwb = nw_b[bb]
        lo_starts.append(lo_c)
        hi_starts.append(hi_c)
        lo_c += nwb * KL * 8
        hi_c += nwb * KH * 8
    CL, CH = lo_c, hi_c

    metas = []
    for c in range(NCORES):
        l, w, half, idx16, win = percore[c]
        wb = win % WPB                              # window within block
        b = win // WPB
        nw = np.array(nw_b)[b]
        # position within the (window, half) group
        grp = win * 2 + half
        gcnt = np.bincount(grp, minlength=2 * NWINDOW)
        gstart = np.cumsum(gcnt) - gcnt
        p_in = np.arange(len(l)) - gstart[grp]
        j = p_in // 128
        row = p_in % 128
        col = np.where(
            half == 0,
            blk_start[b] + wb * KL + j,
            blk_start[b] + nw * KL + wb * KH + j,
        )

        offv = np.zeros((128, TOT), np.float32)
        ewv = np.zeros((128, TOT), np.float32)
        offv[row, col] = (l % WDST).astype(np.float32)
        ewv[row, col] = w

        # gather index lists, one per (block, half), packed along columns.
        # list position i = (col_rel * 128 + row); idx 0 pads (killed by ew=0)
        idxlo = np.zeros((16, CL), np.int16)
        idxhi = np.zeros((16, CH), np.int16)
        for h, (arr, starts, KX) in enumerate(
                [(idxlo, lo_starts, KL), (idxhi, hi_starts, KH)]):
            mh = half == h
            bb = b[mh]
            col_rel = wb[mh] * KX + j[mh]           # position within half
            i_list = col_rel * 128 + row[mh]        # position in block's list
            ci = np.array(starts)[bb] * 16 + i_list  # global flat position
            arr[ci % 16, ci // 16] = idx16[mh]
        metas.append(dict(
            offv=offv,
            ewv=ewv,
            idxlo=np.ascontiguousarray(np.tile(idxlo, (8, 1))),
            idxhi=np.ascontiguousarray(np.tile(idxhi, (8, 1))),
        ))

    # padded CSR of edge weights for the degree computation (no self-loops;
    # the +1 self weight is added on device via the sqrt bias)
    for c in range(NCORES):
        l, w, half, idx16, win = percore[c]
        counts = np.bincount(l, minlength=NPAD)
        starts = np.cumsum(counts) - counts
        o2 = np.argsort(l, kind="stable")
        ls, ws = l[o2], w[o2]
        slot = np.arange(len(ls)) - starts[ls]
        csr = np.zeros((NPAD, KDEG), np.float32)
        csr[ls, slot] = ws
        metas[c]["csr"] = csr

    pre = dict(KL=KL, KH=KH, KDEG=KDEG, TOT=TOT, nw_b=nw_b,
               blk_start=[int(v) for v in blk_start],
               lo_starts=lo_starts, hi_starts=hi_starts, CL=CL, CH=CH)
    return pre, metas


def _to_bf16(a: np.ndarray) -> np.ndarray:
    import ml_dtypes
    return np.ascontiguousarray(a.astype(ml_dtypes.bfloat16))


def make_in_maps(inp: dict, pre, metas):
    iota = np.tile(np.arange(WDST, dtype=np.float32), (128, 1))
    ident = np.eye(128, dtype=np.float32)
    W0 = np.asarray(inp["W0"], np.float32)
    x = np.ascontiguousarray(np.asarray(inp["x"], np.float32))
    shared = dict(
        iota=_to_bf16(iota),
        ident=_to_bf16(ident),
        w0=W0,
        w0t=np.ascontiguousarray(W0.T),
        wiht=np.ascontiguousarray(np.asarray(inp["gru_w_ih"], np.float32).T),
        whht=np.ascontiguousarray(np.asarray(inp["gru_w_hh"], np.float32).T),
        bih=np.asarray(inp["gru_b_ih"], np.float32),
        bhh=np.asarray(inp["gru_b_hh"], np.float32),
        linw=np.ascontiguousarray(np.asarray(inp["lin_w"], np.float32).T),
        linb=np.asarray(inp["lin_b"], np.float32).reshape(1, 1),
    )
    maps = []
    for c in range(NCORES):
        xo = np.zeros((NPAD, C), np.float32)
        xo[:NPC] = x[c * NPC:(c + 1) * NPC]
        m = dict(shared, x_own=xo,
                 offv=metas[c]["offv"], ewv=metas[c]["ewv"],
                 idxlo=metas[c]["idxlo"], idxhi=metas[c]["idxhi"],
                 csr=metas[c]["csr"])
        maps.append(m)
    return maps


# ---------------------------------------------------------------------------
# Device program
# ---------------------------------------------------------------------------

def build_program(pre, skip_collective: bool = False, nblk: int = NBLK,
                  rep: int = 1, nqueues: int = 4):
    import concourse.bacc as bacc
    import concourse.bass as bass
    import concourse.tile as tile
    from concourse import mybir

    f32 = mybir.dt.float32
    bf16 = mybir.dt.bfloat16
    i16 = mybir.dt.int16
    AF = mybir.ActivationFunctionType
    OP = mybir.AluOpType
    KL, KH, KDEG, TOT = pre["KL"], pre["KH"], pre["KDEG"], pre["TOT"]
    nw_b = pre["nw_b"]
    blk_start = pre["blk_start"]
    lo_starts, hi_starts = pre["lo_starts"], pre["hi_starts"]
    CL, CH = pre["CL"], pre["CH"]
    MAXCOL = WPB * (KL + KH)          # widest block in meta columns

    nc = bacc.Bacc("TRN2", target_bir_lowering=False, debug=False,
                   num_devices=NCORES, num_swdge_queues=nqueues)

    x_own_t = nc.declare_dram_parameter("x_own", [NPAD, C], f32, isOutput=False)
    idxlo_t = nc.declare_dram_parameter("idxlo", [128, CL], i16, isOutput=False)
    idxhi_t = nc.declare_dram_parameter("idxhi", [128, CH], i16, isOutput=False)
    offv_t = nc.declare_dram_parameter("offv", [128, TOT], f32, isOutput=False)
    ewv_t = nc.declare_dram_parameter("ewv", [128, TOT], f32, isOutput=False)
    csr_t = nc.declare_dram_parameter("csr", [NPAD, KDEG], f32, isOutput=False)
    iota_t = nc.declare_dram_parameter("iota", [128, WDST], bf16, isOutput=False)
    ident_t = nc.declare_dram_parameter("ident", [128, 128], bf16,
                                        isOutput=False)
    w0_t = nc.declare_dram_parameter("w0", [C, C], f32, isOutput=False)
    w0t_t = nc.declare_dram_parameter("w0t", [C, C], f32, isOutput=False)
    wiht_t = nc.declare_dram_parameter("wiht", [C, 3 * C], f32, isOutput=False)
    whht_t = nc.declare_dram_parameter("whht", [C, 3 * C], f32, isOutput=False)
    bih_t = nc.declare_dram_parameter("bih", [3 * C], f32, isOutput=False)
    bhh_t = nc.declare_dram_parameter("bhh", [3 * C], f32, isOutput=False)
    linw_t = nc.declare_dram_parameter("linw", [C, 1], f32, isOutput=False)
    linb_t = nc.declare_dram_parameter("linb", [1, 1], f32, isOutput=False)
    out_t = nc.declare_dram_parameter("out", [NPAD], f32, isOutput=True)

    xs_paired_hbm = nc.dram_tensor("xs_paired_hbm", [HALFL, 2 * C], bf16)
    xs_all = nc.dram_tensor("xs_all", [TROWS, 2 * C], bf16,
                            addr_space="Shared")

    def bcast_partitions(ap, parts=128):
        return bass.AP(tensor=ap.tensor, offset=ap.offset,
                       ap=[[0, parts]] + list(ap.ap))

    with tile.TileContext(nc) as tc:
        with (
            tc.tile_pool(name="singles", bufs=1) as singles,
            tc.tile_pool(name="gru", bufs=1) as gru,
            tc.tile_pool(name="gpool", bufs=3) as gpool,
            tc.tile_pool(name="spool", bufs=16) as spool,
            tc.tile_pool(name="upool", bufs=3) as upool,
            tc.tile_pool(name="rpool", bufs=3) as rpool,
            tc.tile_pool(name="pagg", bufs=2, space="PSUM") as pagg,
            tc.tile_pool(name="ph", bufs=2, space="PSUM") as ph,
            tc.tile_pool(name="py", bufs=2, space="PSUM") as py,
        ):
            # ---------------- constants / metadata loads ----------------
            iota_sb = singles.tile([128, WDST], bf16)
            nc.sync.dma_start(iota_sb[:], iota_t[:])
            ident_sb = singles.tile([128, 128], bf16)
            nc.sync.dma_start(ident_sb[:], ident_t[:])
            idxlo_sb = singles.tile([128, CL], i16)
            nc.sync.dma_start(idxlo_sb[:], idxlo_t[:])
            idxhi_sb = singles.tile([128, CH], i16)
            nc.sync.dma_start(idxhi_sb[:], idxhi_t[:])
            offv_sb = singles.tile([128, TOT], f32)
            nc.sync.dma_start(offv_sb[:], offv_t[:])
            ewv_sb = singles.tile([128, TOT], f32)
            nc.sync.dma_start(ewv_sb[:], ewv_t[:])
            linw_sb = singles.tile([C, 1], f32)
            nc.sync.dma_start(linw_sb[:], linw_t[:])
            linw_bf = singles.tile([C, 1], bf16)
            nc.scalar.activation(linw_bf[:], linw_sb[:], AF.Copy)
            linb_bc = singles.tile([128, 1], f32)
            nc.gpsimd.dma_start(out=linb_bc[:],
                                in_=bcast_partitions(linb_t[:1, :1]))

            # ---------------- phase A: deg -> dinv -> xs -> allgather ----
            csr_sb = singles.tile([128, NTILE, KDEG], f32)
            nc.sync.dma_start(csr_sb[:],
                              csr_t[:].rearrange("(t r) k -> r t k", r=128))
            deg_sb = singles.tile([128, NTILE], f32)
            for t in range(NTILE):
                nc.vector.reduce_sum(deg_sb[:, t:t + 1], csr_sb[:, t, :],
                                     axis=mybir.AxisListType.X)
            # sqrt(deg + 1): the +1 is the gcn_norm self-loop weight
            sqrt_sb = singles.tile([128, NTILE], f32)
            nc.scalar.activation(sqrt_sb[:], deg_sb[:], AF.Sqrt, bias=1.0)
            dinv_sb = singles.tile([128, NTILE], f32)
            nc.vector.reciprocal(dinv_sb[:], sqrt_sb[:])

            xown_sb = singles.tile([128, NTILE, C], f32)
            nc.sync.dma_start(
                xown_sb[:], x_own_t[:].rearrange("(t r) f -> r t f", r=128))
            xs_sb = singles.tile([128, NTILE, C], bf16)
            for t in range(NTILE):
                nc.vector.tensor_scalar(
                    out=xs_sb[:, t, :], in0=xown_sb[:, t, :],
                    scalar1=dinv_sb[:, t:t + 1], scalar2=None, op0=OP.mult)
            # paired layout: table row r holds local rows r (cols 0:C) and
            # r + HALFL (cols C:2C)
            nhalf = NTILE // 2
            nc.sync.dma_start(
                xs_paired_hbm[:, :C].rearrange("(t r) f -> r t f", r=128),
                xs_sb[:, :nhalf, :])
            nc.sync.dma_start(
                xs_paired_hbm[:, C:].rearrange("(t r) f -> r t f", r=128),
                xs_sb[:, nhalf:, :])
            if skip_collective:
                nc.sync.dma_start(xs_all[:HALFL, :], xs_paired_hbm[:])
            else:
                nc.gpsimd.collective_compute(
                    "AllGather",
                    OP.bypass,
                    replica_groups=[list(range(NCORES))],
                    ins=[xs_paired_hbm[:].opt()],
                    outs=[xs_all[:].opt()],
                )

            # ---------------- GRU weight evolution ----------------------
            w0_sb = gru.tile([C, C], f32)
            nc.sync.dma_start(w0_sb[:], w0_t[:])
            w0t_sb = gru.tile([C, C], f32)
            nc.sync.dma_start(w0t_sb[:], w0t_t[:])
            wiht_sb = gru.tile([C, 3 * C], f32)
            nc.sync.dma_start(wiht_sb[:], wiht_t[:])
            whht_sb = gru.tile([C, 3 * C], f32)
            nc.sync.dma_start(whht_sb[:], whht_t[:])
            bihb_sb = gru.tile([128, 3 * C], f32)
            nc.gpsimd.dma_start(out=bihb_sb[:], in_=bcast_partitions(bih_t[:]))
            bhhb_sb = gru.tile([128, 3 * C], f32)
            nc.gpsimd.dma_start(out=bhhb_sb[:], in_=bcast_partitions(bhh_t[:]))

            gx_ps = pagg.tile([128, BLK], f32, tag="agg_ps")
            nc.tensor.matmul(gx_ps[:, :3 * C], lhsT=w0t_sb[:], rhs=wiht_sb[:],
                             start=True, stop=True)
            gxb = gru.tile([128, 3 * C], f32)
            nc.vector.tensor_tensor(out=gxb[:], in0=gx_ps[:, :3 * C],
                                    in1=bihb_sb[:], op=OP.add)
            gh_ps = pagg.tile([128, BLK], f32, tag="agg_ps")
            nc.tensor.matmul(gh_ps[:, :3 * C], lhsT=w0t_sb[:], rhs=whht_sb[:],
                             start=True, stop=True)
            ghb = gru.tile([128, 3 * C], f32)
            nc.vector.tensor_tensor(out=ghb[:], in0=gh_ps[:, :3 * C],
                                    in1=bhhb_sb[:], op=OP.add)
            rz_in = gru.tile([128, 2 * C], f32)
            nc.vector.tensor_tensor(out=rz_in[:], in0=gxb[:, :2 * C],
                                    in1=ghb[:, :2 * C], op=OP.add)
            rz = gru.tile([128, 2 * C], f32)
            nc.scalar.activation(rz[:], rz_in[:], AF.Sigmoid)
            t1 = gru.tile([128, C], f32)
            nc.vector.tensor_tensor(out=t1[:], in0=rz[:, :C],
                                    in1=ghb[:, 2 * C:], op=OP.mult)
            t2 = gru.tile([128, C], f32)
            nc.vector.tensor_tensor(out=t2[:], in0=gxb[:, 2 * C:], in1=t1[:],
                                    op=OP.add)
            n_sb = gru.tile([128, C], f32)
            nc.scalar.activation(n_sb[:], t2[:], AF.Tanh)
            d_sb = gru.tile([128, C], f32)
            nc.vector.tensor_tensor(out=d_sb[:], in0=w0_sb[:], in1=n_sb[:],
                                    op=OP.subtract)
            e_sb = gru.tile([128, C], f32)
            nc.vector.tensor_tensor(out=e_sb[:], in0=rz[:, C:], in1=d_sb[:],
                                    op=OP.mult)
            w_sb = gru.tile([C, C], f32)
            nc.vector.tensor_tensor(out=w_sb[:], in0=n_sb[:], in1=e_sb[:],
                                    op=OP.add)
            w_bf = gru.tile([C, C], bf16)
            nc.scalar.activation(w_bf[:], w_sb[:], AF.Copy)

            # output accumulator (one dinv-scaled column per window)
            y_sb = singles.tile([128, NTILE], f32)

            # ---------------- phase B: gather + aggregate ----------------
            blk_list = [bb for _ in range(rep) for bb in range(nblk)]
            gtiles = {}

            def issue_block_loads(bi):
                b = blk_list[bi]
                nw = nw_b[b]
                ncol = nw * (KL + KH)
                g_sb = gpool.tile([128, MAXCOL, C], bf16, tag="g")
                nc.gpsimd.dma_gather(
                    g_sb[:, :nw * KL, :],
                    xs_all[:, :C],
                    idxlo_sb[:, lo_starts[b]:lo_starts[b] + nw * KL * 8],
                    nw * KL * 128,
                    nw * KL * 128,
                    C,
                    elem_step=2 * C,
                    single_packet=False,
                    queue_num=(2 * b) % nqueues,
                )
                nc.gpsimd.dma_gather(
                    g_sb[:, nw * KL:ncol, :],
                    xs_all[:, C:],
                    idxhi_sb[:, hi_starts[b]:hi_starts[b] + nw * KH * 8],
                    nw * KH * 128,
                    nw * KH * 128,
                    C,
                    elem_step=2 * C,
                    single_packet=False,
                    queue_num=(2 * b + 1) % nqueues,
                )
                gtiles[bi] = g_sb

            issue_block_loads(0)
            if len(blk_list) > 1:
                issue_block_loads(1)
            for bi, b in enumerate(blk_list):
                nw = nw_b[b]
                c0 = blk_start[b]
                nd = nw * WDST
                g_sb = gtiles.pop(bi)
                if bi + 2 < len(blk_list):
                    issue_block_loads(bi + 2)

                agg_ps = pagg.tile([128, BLK], f32, tag="agg_ps")
                for w in range(nw):
                    gw = WPB * b + w
                    # self-loop: + xs[d] via identity (gcn_norm weight 1)
                    nc.tensor.matmul(
                        agg_ps[:, w * WDST:(w + 1) * WDST],
                        lhsT=xs_sb[:, gw, :],
                        rhs=ident_sb[:],
                        start=True, stop=False,
                    )
                    for h, KX in ((0, KL), (1, KH)):
                        for j in range(KX):
                            col = (w * KL + j) if h == 0 else (
                                nw * KL + w * KH + j)
                            s_sb = spool.tile([128, WDST], bf16, tag="s")
                            nc.vector.tensor_scalar(
                                out=s_sb[:],
                                in0=iota_sb[:],
                                scalar1=offv_sb[:, c0 + col:c0 + col + 1],
                                scalar2=ewv_sb[:, c0 + col:c0 + col + 1],
                                op0=OP.is_equal,
                                op1=OP.mult,
                            )
                            nc.tensor.matmul(
                                agg_ps[:, w * WDST:(w + 1) * WDST],
                                lhsT=g_sb[:, col, :],
                                rhs=s_sb[:],
                                start=False,
                                stop=(h == 1 and j == KH - 1),
                            )

                u_sb = upool.tile([128, BLK], bf16, tag="u")
                nc.scalar.activation(u_sb[:, :nd], agg_ps[:, :nd], AF.Copy)
                h_ps = ph.tile([128, BLK], f32, tag="h")
                nc.tensor.matmul(h_ps[:, :nd], lhsT=w_bf[:], rhs=u_sb[:, :nd],
                                 start=True, stop=True)
                r_sb = rpool.tile([128, BLK], bf16, tag="r")
                nc.scalar.activation(r_sb[:, :nd], h_ps[:, :nd], AF.Relu)
                for w in range(nw):
                    gw = WPB * b + w
                    y_ps = py.tile([128, 1], f32, tag="y")
                    nc.tensor.matmul(y_ps[:, :1],
                                     lhsT=r_sb[:, w * WDST:(w + 1) * WDST],
                                     rhs=linw_bf[:], start=True, stop=True)
                    nc.vector.tensor_scalar(
                        out=y_sb[:, gw:gw + 1], in0=y_ps[:, :1],
                        scalar1=dinv_sb[:, gw:gw + 1],
                        scalar2=linb_bc[:, :1],
                        op0=OP.mult, op1=OP.add)
                nc.sync.dma_start(
                    out_t[BLK * b:BLK * b + nd].rearrange("(t r) -> r t",
                                                          r=128),
                    y_sb[:, WPB * b:WPB * b + nw])
    nc.compile()
    return nc


# ---------------------------------------------------------------------------
# Entry point
# ---------------------------------------------------------------------------

_PROG_CACHE = {}


def kernel(x, edge_index, edge_weight, W0, gru_w_ih, gru_w_hh,
           gru_b_ih, gru_b_hh, lin_w, lin_b):
    from concourse.bass_utils import run_bass_kernel_spmd

    pre, metas = preprocess(np.asarray(edge_index), np.asarray(edge_weight))
    key = (pre["KL"], pre["KH"], pre["KDEG"], pre["TOT"])
    if key not in _PROG_CACHE:
        _PROG_CACHE[key] = build_program(pre)
    nc = _PROG_CACHE[key]
    inp = dict(x=x, W0=W0, gru_w_ih=gru_w_ih, gru_w_hh=gru_w_hh,
               gru_b_ih=gru_b_ih, gru_b_hh=gru_b_hh, lin_w=lin_w, lin_b=lin_b)
    in_maps = make_in_maps(inp, pre, metas)
    res = run_bass_kernel_spmd(nc, in_maps, list(range(NCORES)))
    out = np.concatenate([np.asarray(res.results[c]["out"])[:NPC]
                          for c in range(NCORES)])
    return out.reshape(N_NODES, 1).astype(np.float32)


# revision 7
# speedup vs baseline: 1.5366x; 1.0359x over previous
"""EvolveGCNO RecurrentGCN forward on 8 trn2 NeuronCores.

Strategy (dst-sharded gather, v3 - bf16):
  - Nodes sharded by destination across 8 cores (6250 each, padded to 6400).
    Edges live on the core owning their dst; self-loops are NOT materialized
    as edges - they are applied on-device as identity matmuls over the local
    scaled-feature tiles.
  - Phase A (device): per-core degree via padded-CSR row sums, dinv =
    1/sqrt(deg + 1); scale own x rows by dinv -> xs (bf16); ONE AllGather of
    xs laid out as [3200, 2C] row-pairs (pair = local rows l and l+3200), so
    the full 25600-row table stays int16-indexable and gathers select the
    pair half via a column-offset access pattern.
  - GRU weight evolution on device (replicated on every core).
  - Phase B (device): per block of 512 dsts, dma_gather bf16 xs[src] rows
    (256B descriptors - the memory-roofline term); build one-hot scatter
    matrices S[e, j] = (iota == off) * ew in bf16 (DVE 4x mode) per 128-edge
    chunk; aggregate with bf16 PE matmuls into PSUM [128f, 512d], self-loop
    term added via identity matmuls on xs tiles; then W matmul, ReLU, lin
    row; dinv[dst] is factored past ReLU/lin and applied per-window as a
    [128, 1] per-partition column op.

Host work is limited to graph partitioning / index manipulation / layout
(sorting, bincount, padding, parameter transposes); all floating point math
on tensor values happens on device.
"""

import math
import sys

import numpy as np

sys.path.insert(0, "/opt/trn_rl_repo")

N_NODES, N_EDGES, C = 50000, 600000, 128
NCORES = 8
NPC = N_NODES // NCORES            # 6250 nodes per core
NTILE = 50                         # sbuf tiles of 128 nodes
NPAD = NTILE * 128                 # 6400 padded nodes per core
HALFL = NPAD // 2                  # 3200: per-core pair split
TROWS = NCORES * HALFL             # 25600 rows in the paired gather table
WDST = 128                         # dsts per psum column window
NWINDOW = NPAD // WDST             # 50 windows per core
BLK = 512                          # dsts per psum block
WPB = BLK // WDST                  # full-block windows (4)
NBLK = (NPAD + BLK - 1) // BLK     # 13 blocks (last block: 2 windows)


# ---------------------------------------------------------------------------
# Host-side preprocessing: graph partitioning + layout (index work only)
# ---------------------------------------------------------------------------

def preprocess(edge_index: np.ndarray, edge_weight: np.ndarray):
    src = np.asarray(edge_index[0], dtype=np.int64)
    dst = np.asarray(edge_index[1], dtype=np.int64)
    ew = np.asarray(edge_weight, dtype=np.float32)

    core_of = dst // NPC
    nw_b = [min(WPB, NWINDOW - WPB * b) for b in range(NBLK)]
    percore = []
    kl = kh = 1
    kdeg = 1
    for c in range(NCORES):
        m = core_of == c
        s = src[m]
        l = (dst[m] - c * NPC).astype(np.int64)
        w = ew[m]
        sc = s // NPC
        sl = s - sc * NPC
        half = (sl >= HALFL).astype(np.int64)
        idx16 = (sc * HALFL + (sl - half * HALFL)).astype(np.int64)
        # group edges by (window, half), dst-sorted within each group
        key = (l // WDST) * 2 + half
        order = np.argsort(key * (NPAD + 1) + l, kind="stable")
        l, w, half, idx16 = l[order], w[order], half[order], idx16[order]
        win = l // WDST
        percore.append((l, w, half, idx16, win))
        for h in (0, 1):
            cnt = np.bincount(win[half == h], minlength=NWINDOW)
            k = int(math.ceil(max(int(cnt.max()), 1) / 128))
            if h == 0:
                kl = max(kl, k)
            else:
                kh = max(kh, k)
        kdeg = max(kdeg, int(np.bincount(l, minlength=NPAD).max()))

    KL, KH, KDEG = kl, kh, kdeg
    blk_start = np.cumsum([0] + [nw * (KL + KH) for nw in nw_b])
    TOT = int(blk_start[-1])                       # total meta columns

    lo_starts, hi_starts = [], []
    lo_c = hi_c = 0
    for bb in range(NBLK):
        nwb = nw_b[bb]
        lo_starts.append(lo_c)
        hi_starts.append(hi_c)
        lo_c += nwb * KL * 8
        hi_c += nwb * KH * 8
    CL, CH = lo_c, hi_c

    metas = []
    for c in range(NCORES):
        l, w, half, idx16, win = percore[c]
        wb = win % WPB                              # window within block
        b = win // WPB
        nw = np.array(nw_b)[b]
        # position within the (window, half) group
        grp = win * 2 + half
        gcnt = np.bincount(grp, minlength=2 * NWINDOW)
        gstart = np.cumsum(gcnt) - gcnt
        p_in = np.arange(len(l)) - gstart[grp]
        j = p_in // 128
        row = p_in % 128
        col = np.where(
            half == 0,
            blk_start[b] + wb * KL + j,
            blk_start[b] + nw * KL + wb * KH + j,
        )

        offv = np.zeros((128, TOT), np.float32)
        ewv = np.zeros((128, TOT), np.float32)
        offv[row, col] = (l % WDST).astype(np.float32)
        ewv[row, col] = w

        # gather index lists, one per (block, half), packed along columns.
        # list position i = (col_rel * 128 + row); idx 0 pads (killed by ew=0)
        idxlo = np.zeros((16, CL), np.int16)
        idxhi = np.zeros((16, CH), np.int16)
        for h, (arr, starts, KX) in enumerate(
                [(idxlo, lo_starts, KL), (idxhi, hi_starts, KH)]):
            mh = half == h
            bb = b[mh]
            col_rel = wb[mh] * KX + j[mh]           # position within half
            i_list = col_rel * 128 + row[mh]        # position in block's list
            ci = np.array(starts)[bb] * 16 + i_list  # global flat position
            arr[ci % 16, ci // 16] = idx16[mh]
        metas.append(dict(
            offv=offv,
            ewv=ewv,
            idxlo=np.ascontiguousarray(np.tile(idxlo, (8, 1))),
            idxhi=np.ascontiguousarray(np.tile(idxhi, (8, 1))),
        ))

    # padded CSR of edge weights for the degree computation (no self-loops;
    # the +1 self weight is added on device via the sqrt bias)
    for c in range(NCORES):
        l, w, half, idx16, win = percore[c]
        counts = np.bincount(l, minlength=NPAD)
        starts = np.cumsum(counts) - counts
        o2 = np.argsort(l, kind="stable")
        ls, ws = l[o2], w[o2]
        slot = np.arange(len(ls)) - starts[ls]
        csr = np.zeros((NPAD, KDEG), np.float32)
        csr[ls, slot] = ws
        metas[c]["csr"] = csr

    pre = dict(KL=KL, KH=KH, KDEG=KDEG, TOT=TOT, nw_b=nw_b,
               blk_start=[int(v) for v in blk_start],
               lo_starts=lo_starts, hi_starts=hi_starts, CL=CL, CH=CH)
    return pre, metas


def _to_bf16(a: np.ndarray) -> np.ndarray:
    import ml_dtypes
    return np.ascontiguousarray(a.astype(ml_dtypes.bfloat16))


def make_in_maps(inp: dict, pre, metas):
    iota = np.tile(np.arange(WDST, dtype=np.float32), (128, 1))
    ident = np.eye(128, dtype=np.float32)
    W0 = np.asarray(inp["W0"], np.float32)
    x = np.ascontiguousarray(np.asarray(inp["x"], np.float32))
    shared = dict(
        iota=_to_bf16(iota),
        ident=_to_bf16(ident),
        w0=W0,
        w0t=np.ascontiguousarray(W0.T),
        wiht=np.ascontiguousarray(np.asarray(inp["gru_w_ih"], np.float32).T),
        whht=np.ascontiguousarray(np.asarray(inp["gru_w_hh"], np.float32).T),
        bih=np.asarray(inp["gru_b_ih"], np.float32),
        bhh=np.asarray(inp["gru_b_hh"], np.float32),
        linw=np.ascontiguousarray(np.asarray(inp["lin_w"], np.float32).T),
        linb=np.asarray(inp["lin_b"], np.float32).reshape(1, 1),
    )
    maps = []
    for c in range(NCORES):
        xo = np.zeros((NPAD, C), np.float32)
        xo[:NPC] = x[c * NPC:(c + 1) * NPC]
        m = dict(shared, x_own=xo,
                 offv=metas[c]["offv"], ewv=metas[c]["ewv"],
                 idxlo=metas[c]["idxlo"], idxhi=metas[c]["idxhi"],
                 csr=metas[c]["csr"])
        maps.append(m)
    return maps


# ---------------------------------------------------------------------------
# Device program
# ---------------------------------------------------------------------------

def build_program(pre, skip_collective: bool = False, nblk: int = NBLK,
                  rep: int = 1, nqueues: int = 4):
    import concourse.bacc as bacc
    import concourse.bass as bass
    import concourse.tile as tile
    from concourse import mybir

    f32 = mybir.dt.float32
    bf16 = mybir.dt.bfloat16
    i16 = mybir.dt.int16
    AF = mybir.ActivationFunctionType
    OP = mybir.AluOpType
    KL, KH, KDEG, TOT = pre["KL"], pre["KH"], pre["KDEG"], pre["TOT"]
    nw_b = pre["nw_b"]
    blk_start = pre["blk_start"]
    lo_starts, hi_starts = pre["lo_starts"], pre["hi_starts"]
    CL, CH = pre["CL"], pre["CH"]
    MAXCOL = WPB * (KL + KH)          # widest block in meta columns

    nc = bacc.Bacc("TRN2", target_bir_lowering=False, debug=False,
                   num_devices=NCORES, num_swdge_queues=nqueues)

    x_own_t = nc.declare_dram_parameter("x_own", [NPAD, C], f32, isOutput=False)
    idxlo_t = nc.declare_dram_parameter("idxlo", [128, CL], i16, isOutput=False)
    idxhi_t = nc.declare_dram_parameter("idxhi", [128, CH], i16, isOutput=False)
    offv_t = nc.declare_dram_parameter("offv", [128, TOT], f32, isOutput=False)
    ewv_t = nc.declare_dram_parameter("ewv", [128, TOT], f32, isOutput=False)
    csr_t = nc.declare_dram_parameter("csr", [NPAD, KDEG], f32, isOutput=False)
    iota_t = nc.declare_dram_parameter("iota", [128, WDST], bf16, isOutput=False)
    ident_t = nc.declare_dram_parameter("ident", [128, 128], bf16,
                                        isOutput=False)
    w0_t = nc.declare_dram_parameter("w0", [C, C], f32, isOutput=False)
    w0t_t = nc.declare_dram_parameter("w0t", [C, C], f32, isOutput=False)
    wiht_t = nc.declare_dram_parameter("wiht", [C, 3 * C], f32, isOutput=False)
    whht_t = nc.declare_dram_parameter("whht", [C, 3 * C], f32, isOutput=False)
    bih_t = nc.declare_dram_parameter("bih", [3 * C], f32, isOutput=False)
    bhh_t = nc.declare_dram_parameter("bhh", [3 * C], f32, isOutput=False)
    linw_t = nc.declare_dram_parameter("linw", [C, 1], f32, isOutput=False)
    linb_t = nc.declare_dram_parameter("linb", [1, 1], f32, isOutput=False)
    out_t = nc.declare_dram_parameter("out", [NPAD], f32, isOutput=True)

    xs_paired_hbm = nc.dram_tensor("xs_paired_hbm", [HALFL, 2 * C], bf16)
    xs_all = nc.dram_tensor("xs_all", [TROWS, 2 * C], bf16,
                            addr_space="Shared")

    def bcast_partitions(ap, parts=128):
        return bass.AP(tensor=ap.tensor, offset=ap.offset,
                       ap=[[0, parts]] + list(ap.ap))

    with tile.TileContext(nc) as tc:
        with (
            tc.tile_pool(name="singles", bufs=1) as singles,
            tc.tile_pool(name="gru", bufs=1) as gru,
            tc.tile_pool(name="gpool", bufs=4) as gpool,
            tc.tile_pool(name="spool", bufs=16) as spool,
            tc.tile_pool(name="upool", bufs=3) as upool,
            tc.tile_pool(name="rpool", bufs=3) as rpool,
            tc.tile_pool(name="pagg", bufs=2, space="PSUM") as pagg,
            tc.tile_pool(name="ph", bufs=2, space="PSUM") as ph,
            tc.tile_pool(name="py", bufs=2, space="PSUM") as py,
        ):
            nhalf = NTILE // 2
            # ------------ phase A: deg -> dinv -> xs -> allgather --------
            # Critical-path loads go first on the SP queue: csr (degree),
            # then x in pair-aligned chunks so scaling can start early.
            csr_sb = singles.tile([128, NTILE, KDEG], f32)
            csr_r = csr_t[:].rearrange("(t r) k -> r t k", r=128)
            nc.sync.dma_start(csr_sb[:, :nhalf, :], csr_r[:, :nhalf, :])
            nc.sync.dma_start(csr_sb[:, nhalf:, :], csr_r[:, nhalf:, :])
            xown_sb = singles.tile([128, NTILE, C], f32)
            xown_r = x_own_t[:].rearrange("(t r) f -> r t f", r=128)
            tsplit = 13
            for lo, hi in ((0, tsplit), (nhalf, nhalf + tsplit),
                           (tsplit, nhalf), (nhalf + tsplit, NTILE)):
                nc.sync.dma_start(xown_sb[:, lo:hi, :], xown_r[:, lo:hi, :])

            deg_sb = singles.tile([128, NTILE], f32)
            for t in range(NTILE):
                nc.vector.reduce_sum(deg_sb[:, t:t + 1], csr_sb[:, t, :],
                                     axis=mybir.AxisListType.X)
            # sqrt(deg + 1): the +1 is the gcn_norm self-loop weight
            sqrt_sb = singles.tile([128, NTILE], f32)
            nc.scalar.activation(sqrt_sb[:], deg_sb[:], AF.Sqrt, bias=1.0)
            dinv_sb = singles.tile([128, NTILE], f32)
            nc.vector.reciprocal(dinv_sb[:], sqrt_sb[:])

            # metadata loads ride the idle Activation HWDGE queue, issued
            # after the sqrt so they don't delay the x/csr critical path
            iota_sb = singles.tile([128, WDST], bf16)
            nc.scalar.dma_start(iota_sb[:], iota_t[:])
            ident_sb = singles.tile([128, 128], bf16)
            nc.scalar.dma_start(ident_sb[:], ident_t[:])
            idxlo_sb = singles.tile([128, CL], i16)
            nc.scalar.dma_start(idxlo_sb[:], idxlo_t[:])
            idxhi_sb = singles.tile([128, CH], i16)
            nc.scalar.dma_start(idxhi_sb[:], idxhi_t[:])
            offv_sb = singles.tile([128, TOT], f32)
            nc.scalar.dma_start(offv_sb[:], offv_t[:])
            ewv_sb = singles.tile([128, TOT], f32)
            nc.scalar.dma_start(ewv_sb[:], ewv_t[:])
            linw_sb = singles.tile([C, 1], f32)
            nc.scalar.dma_start(linw_sb[:], linw_t[:])
            linw_bf = singles.tile([C, 1], bf16)
            nc.scalar.activation(linw_bf[:], linw_sb[:], AF.Copy)
            linb_bc = singles.tile([128, 1], f32)
            nc.gpsimd.dma_start(out=linb_bc[:],
                                in_=bcast_partitions(linb_t[:1, :1]))

            # paired layout in SBUF: xs2[:, t, 0:C] = local row block t,
            # xs2[:, t, C:2C] = block t + nhalf, so each stored (partition,
            # tile) run is 512B and the store avoids the small-transfer
            # penalty. Table row r thus holds local rows r and r + HALFL.
            xs2_sb = singles.tile([128, nhalf, 2 * C], bf16)
            xsp_r = xs_paired_hbm[:].rearrange("(t r) f -> r t f", r=128)
            for t0, t1 in ((0, tsplit), (tsplit, nhalf)):
                for t in range(t0, t1):
                    nc.vector.tensor_scalar(
                        out=xs2_sb[:, t, :C], in0=xown_sb[:, t, :],
                        scalar1=dinv_sb[:, t:t + 1], scalar2=None,
                        op0=OP.mult)
                    nc.vector.tensor_scalar(
                        out=xs2_sb[:, t, C:], in0=xown_sb[:, t + nhalf, :],
                        scalar1=dinv_sb[:, t + nhalf:t + nhalf + 1],
                        scalar2=None, op0=OP.mult)
                nc.sync.dma_start(xsp_r[:, t0:t1, :], xs2_sb[:, t0:t1, :])
            if skip_collective:
                nc.sync.dma_start(xs_all[:HALFL, :], xs_paired_hbm[:])
            else:
                nc.gpsimd.collective_compute(
                    "AllGather",
                    OP.bypass,
                    replica_groups=[list(range(NCORES))],
                    ins=[xs_paired_hbm[:].opt()],
                    outs=[xs_all[:].opt()],
                )

            # ---------------- GRU weight evolution ----------------------
            w0_sb = gru.tile([C, C], f32)
            nc.scalar.dma_start(w0_sb[:], w0_t[:])
            w0t_sb = gru.tile([C, C], f32)
            nc.scalar.dma_start(w0t_sb[:], w0t_t[:])
            wiht_sb = gru.tile([C, 3 * C], f32)
            nc.scalar.dma_start(wiht_sb[:], wiht_t[:])
            whht_sb = gru.tile([C, 3 * C], f32)
            nc.scalar.dma_start(whht_sb[:], whht_t[:])
            bihb_sb = gru.tile([128, 3 * C], f32)
            nc.gpsimd.dma_start(out=bihb_sb[:], in_=bcast_partitions(bih_t[:]))
            bhhb_sb = gru.tile([128, 3 * C], f32)
            nc.gpsimd.dma_start(out=bhhb_sb[:], in_=bcast_partitions(bhh_t[:]))

            gx_ps = pagg.tile([128, BLK], f32, tag="agg_ps")
            nc.tensor.matmul(gx_ps[:, :3 * C], lhsT=w0t_sb[:], rhs=wiht_sb[:],
                             start=True, stop=True)
            gxb = gru.tile([128, 3 * C], f32)
            nc.vector.tensor_tensor(out=gxb[:], in0=gx_ps[:, :3 * C],
                                    in1=bihb_sb[:], op=OP.add)
            gh_ps = pagg.tile([128, BLK], f32, tag="agg_ps")
            nc.tensor.matmul(gh_ps[:, :3 * C], lhsT=w0t_sb[:], rhs=whht_sb[:],
                             start=True, stop=True)
            ghb = gru.tile([128, 3 * C], f32)
            nc.vector.tensor_tensor(out=ghb[:], in0=gh_ps[:, :3 * C],
                                    in1=bhhb_sb[:], op=OP.add)
            rz_in = gru.tile([128, 2 * C], f32)
            nc.vector.tensor_tensor(out=rz_in[:], in0=gxb[:, :2 * C],
                                    in1=ghb[:, :2 * C], op=OP.add)
            rz = gru.tile([128, 2 * C], f32)
            nc.scalar.activation(rz[:], rz_in[:], AF.Sigmoid)
            t1 = gru.tile([128, C], f32)
            nc.vector.tensor_tensor(out=t1[:], in0=rz[:, :C],
                                    in1=ghb[:, 2 * C:], op=OP.mult)
            t2 = gru.tile([128, C], f32)
            nc.vector.tensor_tensor(out=t2[:], in0=gxb[:, 2 * C:], in1=t1[:],
                                    op=OP.add)
            n_sb = gru.tile([128, C], f32)
            nc.scalar.activation(n_sb[:], t2[:], AF.Tanh)
            d_sb = gru.tile([128, C], f32)
            nc.vector.tensor_tensor(out=d_sb[:], in0=w0_sb[:], in1=n_sb[:],
                                    op=OP.subtract)
            e_sb = gru.tile([128, C], f32)
            nc.vector.tensor_tensor(out=e_sb[:], in0=rz[:, C:], in1=d_sb[:],
                                    op=OP.mult)
            w_sb = gru.tile([C, C], f32)
            nc.vector.tensor_tensor(out=w_sb[:], in0=n_sb[:], in1=e_sb[:],
                                    op=OP.add)
            w_bf = gru.tile([C, C], bf16)
            nc.scalar.activation(w_bf[:], w_sb[:], AF.Copy)

            # output accumulator (one dinv-scaled column per window)
            y_sb = singles.tile([128, NTILE], f32)

            # ---------------- phase B: gather + aggregate ----------------
            blk_list = [bb for _ in range(rep) for bb in range(nblk)]
            gtiles = {}

            def issue_block_loads(bi):
                b = blk_list[bi]
                nw = nw_b[b]
                ncol = nw * (KL + KH)
                g_sb = gpool.tile([128, MAXCOL, C], bf16, tag="g")
                nc.gpsimd.dma_gather(
                    g_sb[:, :nw * KL, :],
                    xs_all[:, :C],
                    idxlo_sb[:, lo_starts[b]:lo_starts[b] + nw * KL * 8],
                    nw * KL * 128,
                    nw * KL * 128,
                    C,
                    elem_step=2 * C,
                    single_packet=False,
                    queue_num=(2 * b) % nqueues,
                )
                nc.gpsimd.dma_gather(
                    g_sb[:, nw * KL:ncol, :],
                    xs_all[:, C:],
                    idxhi_sb[:, hi_starts[b]:hi_starts[b] + nw * KH * 8],
                    nw * KH * 128,
                    nw * KH * 128,
                    C,
                    elem_step=2 * C,
                    single_packet=False,
                    queue_num=(2 * b + 1) % nqueues,
                )
                gtiles[bi] = g_sb

            for pf in range(min(3, len(blk_list))):
                issue_block_loads(pf)
            for bi, b in enumerate(blk_list):
                nw = nw_b[b]
                c0 = blk_start[b]
                nd = nw * WDST
                g_sb = gtiles.pop(bi)
                if bi + 3 < len(blk_list):
                    issue_block_loads(bi + 3)

                agg_ps = pagg.tile([128, BLK], f32, tag="agg_ps")
                for w in range(nw):
                    gw = WPB * b + w
                    xst = (xs2_sb[:, gw, :C] if gw < NTILE // 2
                           else xs2_sb[:, gw - NTILE // 2, C:])
                    # self-loop: + xs[d] via identity (gcn_norm weight 1)
                    nc.tensor.matmul(
                        agg_ps[:, w * WDST:(w + 1) * WDST],
                        lhsT=xst,
                        rhs=ident_sb[:],
                        start=True, stop=False,
                    )
                    for h, KX in ((0, KL), (1, KH)):
                        for j in range(KX):
                            col = (w * KL + j) if h == 0 else (
                                nw * KL + w * KH + j)
                            s_sb = spool.tile([128, WDST], bf16, tag="s")
                            nc.vector.tensor_scalar(
                                out=s_sb[:],
                                in0=iota_sb[:],
                                scalar1=offv_sb[:, c0 + col:c0 + col + 1],
                                scalar2=ewv_sb[:, c0 + col:c0 + col + 1],
                                op0=OP.is_equal,
                                op1=OP.mult,
                            )
                            nc.tensor.matmul(
                                agg_ps[:, w * WDST:(w + 1) * WDST],
                                lhsT=g_sb[:, col, :],
                                rhs=s_sb[:],
                                start=False,
                                stop=(h == 1 and j == KH - 1),
                            )

                u_sb = upool.tile([128, BLK], bf16, tag="u")
                nc.scalar.activation(u_sb[:, :nd], agg_ps[:, :nd], AF.Copy)
                h_ps = ph.tile([128, BLK], f32, tag="h")
                nc.tensor.matmul(h_ps[:, :nd], lhsT=w_bf[:], rhs=u_sb[:, :nd],
                                 start=True, stop=True)
                r_sb = rpool.tile([128, BLK], bf16, tag="r")
                nc.scalar.activation(r_sb[:, :nd], h_ps[:, :nd], AF.Relu)
                for w in range(nw):
                    gw = WPB * b + w
                    y_ps = py.tile([128, 1], f32, tag="y")
                    nc.tensor.matmul(y_ps[:, :1],
                                     lhsT=r_sb[:, w * WDST:(w + 1) * WDST],
                                     rhs=linw_bf[:], start=True, stop=True)
                    nc.vector.tensor_scalar(
                        out=y_sb[:, gw:gw + 1], in0=y_ps[:, :1],
                        scalar1=dinv_sb[:, gw:gw + 1],
                        scalar2=linb_bc[:, :1],
                        op0=OP.mult, op1=OP.add)
                nc.sync.dma_start(
                    out_t[BLK * b:BLK * b + nd].rearrange("(t r) -> r t",
                                                          r=128),
                    y_sb[:, WPB * b:WPB * b + nw])
    nc.compile()
    return nc


# ---------------------------------------------------------------------------
# Entry point
# ---------------------------------------------------------------------------

_PROG_CACHE = {}


def kernel(x, edge_index, edge_weight, W0, gru_w_ih, gru_w_hh,
           gru_b_ih, gru_b_hh, lin_w, lin_b):
    from concourse.bass_utils import run_bass_kernel_spmd

    pre, metas = preprocess(np.asarray(edge_index), np.asarray(edge_weight))
    key = (pre["KL"], pre["KH"], pre["KDEG"], pre["TOT"])
    if key not in _PROG_CACHE:
        _PROG_CACHE[key] = build_program(pre)
    nc = _PROG_CACHE[key]
    inp = dict(x=x, W0=W0, gru_w_ih=gru_w_ih, gru_w_hh=gru_w_hh,
               gru_b_ih=gru_b_ih, gru_b_hh=gru_b_hh, lin_w=lin_w, lin_b=lin_b)
    in_maps = make_in_maps(inp, pre, metas)
    res = run_bass_kernel_spmd(nc, in_maps, list(range(NCORES)))
    out = np.concatenate([np.asarray(res.results[c]["out"])[:NPC]
                          for c in range(NCORES)])
    return out.reshape(N_NODES, 1).astype(np.float32)
